# revision 4
# baseline (speedup 1.0000x reference)
"""Trainium2 Bass kernel for nn_DSRLossStateless (DSR loss, stateless).

loss = -sum_t(D_t)/B where D_t comes from an eta-EMA pair (A,B) over
portfolio returns R_t = sum_a w[t,a]*nr[t,a].

v3 strategy (8 cores, batch-sharded, interleaved layout):
  - Host: fp16 inputs; core m owns 250k rows + 2000 preceding rows
    (history; synthetic EPS-seed rows for core 0). Rows are interleaved
    so SBUF partition p holds times t === p (mod 125): tile [125, 2016]
    with t = col*125 + p. All per-partition DMA stays contiguous.
  - Device, per column-chunk (4 chunks x 504 cols):
      product w*nr (fp16, DVE 2x mode) -> pairwise tree-sum to R (fp16)
      -> R2 = Square(R) on ACT
      -> PE matmuls with triangular geometric matrices L',U' compute the
         125-tap shifted FIR: Y'[p,n] = eta * sum_{j>=1} c^(j-1) R[t-j]
         contributions within the column, accumulated f32 in PSUM
      -> DVE scan with decay c^125 along columns turns Y' into exact
         A_prev[t], B_prev[t] (no carries, no tail correction)
      -> D-chain (f32, split DVE/ACT/GpSimd) accumulates
         q = negn/var^1.5 per chunk, overlapped with the DMA stream.
  - Final: per-partition q sums -> partition gather -> scalar out.
    Host: loss = eta * sum(core outs) / B.
"""

import sys

sys.path.insert(0, "/opt/trn_rl_repo")

import numpy as np

import concourse.bass as bass
import concourse.bacc as bacc
import concourse.tile as tile
from concourse import mybir
from concourse.bass_utils import run_bass_kernel_spmd
from contextlib import ExitStack

F32 = mybir.dt.float32
F16 = mybir.dt.float16
NF32 = np.float32
NF16 = np.float16

N_CORES = 8
NA = 16                 # assets
KP = 125                # partitions (time interleave stride)
PRE_COLS = 16           # prepend history columns
NC_COLS = 2000          # owned columns
COLS = NC_COLS + PRE_COLS   # 2016
OWN = KP * NC_COLS      # 250000 rows per core
PRE = KP * PRE_COLS     # 2000 history rows
B_TOTAL = N_CORES * OWN
CH = 4                  # column chunks
TC = COLS // CH         # 504 columns per chunk
FW = TC * NA            # 8064 fp16 elems per partition per chunk
ETA = 0.01
EPS = 1e-8
CDEC = NF32(1.0 - ETA)  # 0.99
C125 = NF32(float(CDEC) ** KP)

AL = mybir.AluOpType
AF = mybir.ActivationFunctionType
AX = mybir.AxisListType

_PROGRAM = None


def _fir_matrices():
    """lhsT-layout [k, m] FIR matrices with eta folded in.

    A_prev[m, n] = c125*A_prev[m, n-1] + sum_k L[k,m] R[k,n] + U[k,m] R[k,n-1]
    where L[k,m] = eta*c^(m-1-k) for k <= m-1, U[k,m] = eta*c^(125+m-1-k)
    for k >= m.
    """
    k = np.arange(KP)
    m = np.arange(KP)
    c = float(CDEC)
    Lm = np.where(k[:, None] <= m[None, :] - 1,
                  c ** (m[None, :] - 1 - k[:, None]), 0.0)
    Um = np.where(k[:, None] >= m[None, :],
                  c ** (KP + m[None, :] - 1 - k[:, None]), 0.0)
    return (ETA * Lm).astype(NF16), (ETA * Um).astype(NF16)


def _build_program():
    nc = bacc.Bacc("TRN2", target_bir_lowering=False, debug=False)

    w_ap = nc.dram_tensor("w", [KP, COLS * NA], F16, kind="ExternalInput").ap()
    nr_ap = nc.dram_tensor("nr", [KP, COLS * NA], F16, kind="ExternalInput").ap()
    out_ap = nc.dram_tensor("out", [1, 1], F32, kind="ExternalOutput").ap()

    Lnp, Unp = _fir_matrices()
    L_dram = nc.inline_tensor(np.ascontiguousarray(Lnp), name="Lfir")
    U_dram = nc.inline_tensor(np.ascontiguousarray(Unp), name="Ufir")

    with tile.TileContext(nc) as tc, ExitStack() as ctx:
        pers = ctx.enter_context(tc.tile_pool(name="pers", bufs=1))
        loadp = ctx.enter_context(tc.tile_pool(name="load", bufs=2))
        prodp = ctx.enter_context(tc.tile_pool(name="prod", bufs=2))
        dchp = ctx.enter_context(tc.tile_pool(name="dch", bufs=2))
        psump = ctx.enter_context(tc.psum_pool(name="ps", bufs=2))

        Lt = pers.tile([KP, KP], F16, tag="Lt")
        Ut = pers.tile([KP, KP], F16, tag="Ut")
        R = pers.tile([KP, 1 + COLS], F16, tag="R")
        R2 = pers.tile([KP, 1 + COLS], F16, tag="R2")
        Aprev = pers.tile([KP, COLS], F32, tag="Aprev")
        Bprev = pers.tile([KP, COLS], F32, tag="Bprev")
        CD = pers.tile([KP, TC], F32, tag="CD")
        qs = pers.tile([KP, CH], F32, tag="qs")
        qp = pers.tile([KP, 1], F32, tag="qp")
        qrow = pers.tile([1, KP], F32, tag="qrow")
        qtot = pers.tile([1, 1], F32, tag="qtot")

        nc.sync.dma_start(Lt[:], L_dram.ap())
        nc.sync.dma_start(Ut[:], U_dram.ap())
        nc.gpsimd.memset(CD[:, :], float(C125))
        nc.gpsimd.memset(R[:, 0:1], 0.0)
        nc.gpsimd.memset(R2[:, 0:1], 0.0)
        # pin ACT tables (Square/Sqrt/Copy) before the pipeline
        nc.gpsimd.memset(qtot[0:1, 0:1], 1.0)
        nc.scalar.activation(qtot[0:1, 0:1], qtot[0:1, 0:1], AF.Sqrt)

        for k in range(CH):
            ks, ke = k * TC, (k + 1) * TC
            rs = slice(1 + ks, 1 + ke)   # R/R2 slices (col 0 is the pad)
            sh = slice(ks, ke)           # shifted R/R2 slices
            cs = slice(ks, ke)           # Aprev/Bprev/D-chain slices

            wt = loadp.tile([KP, FW], F16, tag="wt")
            rt = loadp.tile([KP, FW], F16, tag="rt")
            nc.sync.dma_start(wt[:], w_ap[:, ks * NA:ke * NA])
            nc.scalar.dma_start(rt[:], nr_ap[:, ks * NA:ke * NA])

            # product + pairwise tree rowsum (fp16, DVE 2x on packed lanes)
            prod = prodp.tile([KP, FW], F16, tag="prod")
            u1 = prodp.tile([KP, TC * 8], F16, tag="u1")
            nc.vector.tensor_mul(prod[:], wt[:], rt[:])
            p3 = prod[:].rearrange("p (t a) -> p t a", a=16)
            u1v = u1[:].rearrange("p (t a) -> p t a", a=8)
            nc.vector.tensor_add(u1v[:, :, :], p3[:, :, 0:8], p3[:, :, 8:16])
            nc.vector.tensor_add(u1v[:, :, 0:4], u1v[:, :, 0:4], u1v[:, :, 4:8])
            nc.vector.tensor_add(u1v[:, :, 0:2], u1v[:, :, 0:2], u1v[:, :, 2:4])
            nc.vector.tensor_add(R[:, rs], u1v[:, :, 0], u1v[:, :, 1])
            nc.scalar.activation(R2[:, rs], R[:, rs], AF.Square)

            # FIR matmuls into PSUM (f32 accumulate)
            YA = psump.tile([KP, TC], F32, tag="YA")
            YB = psump.tile([KP, TC], F32, tag="YB")
            nc.tensor.matmul(YA[:], lhsT=Lt[:], rhs=R[:, rs], start=True, stop=False)
            nc.tensor.matmul(YB[:], lhsT=Lt[:], rhs=R2[:, rs], start=True, stop=False)
            nc.tensor.matmul(YA[:], lhsT=Ut[:], rhs=R[:, sh], start=False, stop=True)
            nc.tensor.matmul(YB[:], lhsT=Ut[:], rhs=R2[:, sh], start=False, stop=True)

            # exact A_prev/B_prev via c^125-decay scans along columns
            initA = 0.0 if k == 0 else Aprev[:, ks - 1:ks]
            initB = 0.0 if k == 0 else Bprev[:, ks - 1:ks]
            nc.vector.tensor_tensor_scan(
                out=Aprev[:, cs], data0=CD[:, :], data1=YA[:],
                initial=initA, op0=AL.mult, op1=AL.add,
            )
            nc.vector.tensor_tensor_scan(
                out=Bprev[:, cs], data0=CD[:, :], data1=YB[:],
                initial=initB, op0=AL.mult, op1=AL.add,
            )

            # D-chain: q = [0.5*A*(R^2+B) - B*R] / var^1.5
            g1 = dchp.tile([KP, TC], F32, tag="g1")
            g3 = dchp.tile([KP, TC], F32, tag="g3")
            t1 = dchp.tile([KP, TC], F32, tag="t1")
            a2 = dchp.tile([KP, TC], F32, tag="a2")
            vv = dchp.tile([KP, TC], F32, tag="vv")
            ss = dchp.tile([KP, TC], F32, tag="ss")
            dd = dchp.tile([KP, TC], F32, tag="dd")
            rc = dchp.tile([KP, TC], F32, tag="rc")
            nc.gpsimd.tensor_add(g1[:], R2[:, rs], Bprev[:, cs])
            nc.gpsimd.tensor_mul(g3[:], Bprev[:, cs], R[:, rs])
            nc.vector.scalar_tensor_tensor(
                out=t1[:], in0=Aprev[:, cs], scalar=0.5, in1=g1[:],
                op0=AL.mult, op1=AL.mult,
            )
            nc.gpsimd.tensor_sub(t1[:], t1[:], g3[:])          # negn
            nc.scalar.activation(a2[:], Aprev[:, cs], AF.Square)
            nc.gpsimd.tensor_sub(vv[:], Bprev[:, cs], a2[:])
            nc.vector.tensor_scalar_max(vv[:], vv[:], EPS)     # var
            nc.scalar.activation(ss[:], vv[:], AF.Sqrt)
            nc.gpsimd.tensor_mul(dd[:], vv[:], ss[:])          # var^1.5
            nc.vector.reciprocal_approx_accurate(rc[:], dd[:], ss[:])
            qa = slice(PRE_COLS, TC) if k == 0 else slice(0, TC)
            nc.vector.scalar_tensor_tensor(
                out=dd[:, qa], in0=t1[:, qa], scalar=1.0, in1=rc[:, qa],
                op0=AL.mult, op1=AL.mult, accum_out=qs[:, k:k + 1],
            )

        # final reduction: per-partition partials -> scalar
        nc.vector.reduce_sum(qp[:, 0:1], qs[:, :], axis=AX.X)
        nc.sync.dma_start(qrow[0:1, 0:KP], qp[0:KP, 0:1])
        nc.vector.reduce_sum(qtot[0:1, 0:1], qrow[0:1, 0:KP], axis=AX.X)
        nc.sync.dma_start(out_ap[0:1, 0:1], qtot[0:1, 0:1])

    nc.compile()
    return nc


def _get_program():
    global _PROGRAM
    if _PROGRAM is None:
        _PROGRAM = _build_program()
    return _PROGRAM


def _core0_prepend():
    """2000 synthetic history rows encoding the global init (A,B)=(0,EPS).

    All-zero rows leave the scan at (0,0); the last two rows carry returns
    r1, r2 with r2 = -c*r1 so the A contribution cancels to ~0, while
    eta*(c*r1^2 + r2^2) ~ EPS supplies the B seed.
    """
    w = np.zeros((PRE, NA), NF32)
    nr = np.zeros((PRE, NA), NF32)
    c = CDEC
    r1 = NF32(np.sqrt(EPS / (ETA * (float(c) + float(c) ** 2))))
    r2 = NF32(-(c * r1))
    w[PRE - 2, 0] = NF32(1.0)
    nr[PRE - 2, 0] = r1
    w[PRE - 1, 0] = NF32(1.0)
    nr[PRE - 1, 0] = r2
    return w, nr


def _interleave16(arr):
    # [COLS*KP, NA] f32 -> [KP, COLS*NA] fp16, t = col*KP + p
    return np.ascontiguousarray(
        arr.reshape(COLS, KP, NA).transpose(1, 0, 2).astype(NF16)
    ).reshape(KP, COLS * NA)


def _make_in_maps(weights, nr):
    weights = np.asarray(weights, dtype=NF32)
    nr = np.asarray(nr, dtype=NF32)
    pre_w, pre_nr = _core0_prepend()
    in_maps = []
    for m in range(N_CORES):
        s = m * OWN
        if m == 0:
            wm = np.concatenate([pre_w, weights[:OWN]])
            rm = np.concatenate([pre_nr, nr[:OWN]])
        else:
            wm = weights[s - PRE:s + OWN]
            rm = nr[s - PRE:s + OWN]
        in_maps.append({"w": _interleave16(wm), "nr": _interleave16(rm)})
    return in_maps


def _run(in_maps, **kwargs):
    nc = _get_program()
    return run_bass_kernel_spmd(nc, in_maps, core_ids=list(range(N_CORES)), **kwargs)


def kernel(weights, next_returns):
    in_maps = _make_in_maps(weights, next_returns)
    res = _run(in_maps)
    total = np.sum(
        np.array([res.results[m]["out"][0, 0] for m in range(N_CORES)], NF32),
        dtype=NF32,
    )
    return NF32(NF32(ETA) * total / NF32(B_TOTAL))


# revision 5
# speedup vs baseline: 1.3817x; 1.3817x over previous
"""Trainium2 Bass kernel for nn_DSRLossStateless (DSR loss, stateless).

loss = -sum_t(D_t)/B where D_t comes from an eta-EMA pair (A,B) over
portfolio returns R_t = sum_a w[t,a]*nr[t,a].

v3 strategy (8 cores, batch-sharded, interleaved layout):
  - Host: fp16 inputs; core m owns 250k rows + 2000 preceding rows
    (history; synthetic EPS-seed rows for core 0). Rows are interleaved
    so SBUF partition p holds times t === p (mod 125): tile [125, 2016]
    with t = col*125 + p. All per-partition DMA stays contiguous.
  - Device, per column-chunk (4 chunks x 504 cols):
      product w*nr (fp16, DVE 2x mode) -> pairwise tree-sum to R (fp16)
      -> R2 = Square(R) on ACT
      -> PE matmuls with triangular geometric matrices L',U' compute the
         125-tap shifted FIR: Y'[p,n] = eta * sum_{j>=1} c^(j-1) R[t-j]
         contributions within the column, accumulated f32 in PSUM
      -> DVE scan with decay c^125 along columns turns Y' into exact
         A_prev[t], B_prev[t] (no carries, no tail correction)
      -> D-chain (f32, split DVE/ACT/GpSimd) accumulates
         q = negn/var^1.5 per chunk, overlapped with the DMA stream.
  - Final: per-partition q sums -> partition gather -> scalar out.
    Host: loss = eta * sum(core outs) / B.
"""

import sys

sys.path.insert(0, "/opt/trn_rl_repo")

import numpy as np

import concourse.bass as bass
import concourse.bacc as bacc
import concourse.tile as tile
from concourse import mybir
from concourse.bass_utils import run_bass_kernel_spmd
from contextlib import ExitStack

F32 = mybir.dt.float32
F16 = mybir.dt.float16
NF32 = np.float32
NF16 = np.float16

N_CORES = 8
NA = 16                 # assets
KP = 125                # partitions (time interleave stride)
PRE_COLS = 16           # prepend history columns
NC_COLS = 2000          # owned columns
COLS = NC_COLS + PRE_COLS   # 2016
OWN = KP * NC_COLS      # 250000 rows per core
PRE = KP * PRE_COLS     # 2000 history rows
B_TOTAL = N_CORES * OWN
CH = 4                  # column chunks
TC = COLS // CH         # 504 columns per chunk
FW = TC * NA            # 8064 fp16 elems per partition per chunk
ETA = 0.01
EPS = 1e-8
CDEC = NF32(1.0 - ETA)  # 0.99
C125 = NF32(float(CDEC) ** KP)

AL = mybir.AluOpType
AF = mybir.ActivationFunctionType
AX = mybir.AxisListType

_PROGRAM = None


def _fir_matrices():
    """lhsT-layout [k, m] FIR matrices with eta folded in.

    A_prev[m, n] = c125*A_prev[m, n-1] + sum_k L[k,m] R[k,n] + U[k,m] R[k,n-1]
    where L[k,m] = eta*c^(m-1-k) for k <= m-1, U[k,m] = eta*c^(125+m-1-k)
    for k >= m.
    """
    k = np.arange(KP)
    m = np.arange(KP)
    c = float(CDEC)
    Lm = np.where(k[:, None] <= m[None, :] - 1,
                  c ** (m[None, :] - 1 - k[:, None]), 0.0)
    Um = np.where(k[:, None] >= m[None, :],
                  c ** (KP + m[None, :] - 1 - k[:, None]), 0.0)
    return (ETA * Lm).astype(NF16), (ETA * Um).astype(NF16)


def _build_program():
    nc = bacc.Bacc("TRN2", target_bir_lowering=False, debug=False)

    # loads span 126 partitions (row 125 is zero padding): 126-row transfers
    # spread across 14 DMA engines; 125-row ones collapse to 5.
    w_ap = nc.dram_tensor("w", [KP + 1, COLS * NA], F16, kind="ExternalInput").ap()
    nr_ap = nc.dram_tensor("nr", [KP + 1, COLS * NA], F16, kind="ExternalInput").ap()
    out_ap = nc.dram_tensor("out", [1, 1], F32, kind="ExternalOutput").ap()

    Lnp, Unp = _fir_matrices()
    L_dram = nc.inline_tensor(np.ascontiguousarray(Lnp), name="Lfir")
    U_dram = nc.inline_tensor(np.ascontiguousarray(Unp), name="Ufir")

    with tile.TileContext(nc) as tc, ExitStack() as ctx:
        pers = ctx.enter_context(tc.tile_pool(name="pers", bufs=1))
        queues = [nc.sync, nc.scalar, nc.gpsimd]
        loadp = ctx.enter_context(tc.tile_pool(name="load", bufs=2))
        prodp = ctx.enter_context(tc.tile_pool(name="prod", bufs=2))
        dchp = ctx.enter_context(tc.tile_pool(name="dch", bufs=2))
        psump = ctx.enter_context(tc.psum_pool(name="ps", bufs=2))

        Lt = pers.tile([KP, KP], F16, tag="Lt")
        Ut = pers.tile([KP, KP], F16, tag="Ut")
        R = pers.tile([KP, 1 + COLS], F16, tag="R")
        R2 = pers.tile([KP, 1 + COLS], F16, tag="R2")
        Aprev = pers.tile([KP, COLS], F32, tag="Aprev")
        Bprev = pers.tile([KP, COLS], F32, tag="Bprev")
        CD = pers.tile([KP, TC], F32, tag="CD")
        qs = pers.tile([KP, CH], F32, tag="qs")
        qp = pers.tile([KP, 1], F32, tag="qp")
        qrow = pers.tile([1, KP], F32, tag="qrow")
        qtot = pers.tile([1, 1], F32, tag="qtot")

        nc.sync.dma_start(Lt[:], L_dram.ap())
        nc.sync.dma_start(Ut[:], U_dram.ap())
        nc.gpsimd.memset(CD[:, :], float(C125))
        nc.gpsimd.memset(R[:, 0:1], 0.0)
        nc.gpsimd.memset(R2[:, 0:1], 0.0)
        # pin ACT tables (Square/Sqrt/Copy) before the pipeline
        nc.gpsimd.memset(qtot[0:1, 0:1], 1.0)
        nc.scalar.activation(qtot[0:1, 0:1], qtot[0:1, 0:1], AF.Sqrt)

        for k in range(CH):
            ks, ke = k * TC, (k + 1) * TC
            rs = slice(1 + ks, 1 + ke)   # R/R2 slices (col 0 is the pad)
            sh = slice(ks, ke)           # shifted R/R2 slices
            cs = slice(ks, ke)           # Aprev/Bprev/D-chain slices

            wt = loadp.tile([KP + 1, FW], F16, tag="wt")
            rt = loadp.tile([KP + 1, FW], F16, tag="rt")
            queues[(2 * k) % 3].dma_start(wt[:], w_ap[:, ks * NA:ke * NA])
            queues[(2 * k + 1) % 3].dma_start(rt[:], nr_ap[:, ks * NA:ke * NA])

            # product (in-place) + pairwise tree rowsum (fp16, DVE 2x)
            u1 = prodp.tile([KP, TC * 8], F16, tag="u1")
            nc.vector.tensor_mul(wt[0:KP, :], wt[0:KP, :], rt[0:KP, :])
            p3 = wt[0:KP, :].rearrange("p (t a) -> p t a", a=16)
            u1v = u1[:].rearrange("p (t a) -> p t a", a=8)
            nc.vector.tensor_add(u1v[:, :, :], p3[:, :, 0:8], p3[:, :, 8:16])
            nc.vector.tensor_add(u1v[:, :, 0:4], u1v[:, :, 0:4], u1v[:, :, 4:8])
            nc.vector.tensor_add(u1v[:, :, 0:2], u1v[:, :, 0:2], u1v[:, :, 2:4])
            nc.vector.tensor_add(R[:, rs], u1v[:, :, 0], u1v[:, :, 1])
            nc.scalar.activation(R2[:, rs], R[:, rs], AF.Square)

            # FIR matmuls into PSUM (f32 accumulate)
            YA = psump.tile([KP, TC], F32, tag="YA")
            YB = psump.tile([KP, TC], F32, tag="YB")
            nc.tensor.matmul(YA[:], lhsT=Lt[:], rhs=R[:, rs], start=True, stop=False)
            nc.tensor.matmul(YB[:], lhsT=Lt[:], rhs=R2[:, rs], start=True, stop=False)
            nc.tensor.matmul(YA[:], lhsT=Ut[:], rhs=R[:, sh], start=False, stop=True)
            nc.tensor.matmul(YB[:], lhsT=Ut[:], rhs=R2[:, sh], start=False, stop=True)

            # exact A_prev/B_prev via c^125-decay scans along columns
            initA = 0.0 if k == 0 else Aprev[:, ks - 1:ks]
            initB = 0.0 if k == 0 else Bprev[:, ks - 1:ks]
            nc.vector.tensor_tensor_scan(
                out=Aprev[:, cs], data0=CD[:, :], data1=YA[:],
                initial=initA, op0=AL.mult, op1=AL.add,
            )
            nc.vector.tensor_tensor_scan(
                out=Bprev[:, cs], data0=CD[:, :], data1=YB[:],
                initial=initB, op0=AL.mult, op1=AL.add,
            )

            # D-chain: q = [0.5*A*(R^2+B) - B*R] / var^1.5
            g1 = dchp.tile([KP, TC], F32, tag="g1")
            g3 = dchp.tile([KP, TC], F32, tag="g3")
            t1 = dchp.tile([KP, TC], F32, tag="t1")
            a2 = dchp.tile([KP, TC], F32, tag="a2")
            vv = dchp.tile([KP, TC], F32, tag="vv")
            ss = dchp.tile([KP, TC], F32, tag="ss")
            dd = dchp.tile([KP, TC], F32, tag="dd")
            rc = dchp.tile([KP, TC], F32, tag="rc")
            nc.gpsimd.tensor_add(g1[:], R2[:, rs], Bprev[:, cs])
            nc.gpsimd.tensor_mul(g3[:], Bprev[:, cs], R[:, rs])
            nc.vector.scalar_tensor_tensor(
                out=t1[:], in0=Aprev[:, cs], scalar=0.5, in1=g1[:],
                op0=AL.mult, op1=AL.mult,
            )
            nc.gpsimd.tensor_sub(t1[:], t1[:], g3[:])          # negn
            nc.scalar.activation(a2[:], Aprev[:, cs], AF.Square)
            nc.gpsimd.tensor_sub(vv[:], Bprev[:, cs], a2[:])
            nc.vector.tensor_scalar_max(vv[:], vv[:], EPS)     # var
            nc.scalar.activation(ss[:], vv[:], AF.Sqrt)
            nc.gpsimd.tensor_mul(dd[:], vv[:], ss[:])          # var^1.5
            nc.vector.reciprocal_approx_accurate(rc[:], dd[:], ss[:])
            qa = slice(PRE_COLS, TC) if k == 0 else slice(0, TC)
            nc.vector.scalar_tensor_tensor(
                out=dd[:, qa], in0=t1[:, qa], scalar=1.0, in1=rc[:, qa],
                op0=AL.mult, op1=AL.mult, accum_out=qs[:, k:k + 1],
            )

        # final reduction: per-partition partials -> scalar
        nc.vector.reduce_sum(qp[:, 0:1], qs[:, :], axis=AX.X)
        nc.sync.dma_start(qrow[0:1, 0:KP], qp[0:KP, 0:1])
        nc.vector.reduce_sum(qtot[0:1, 0:1], qrow[0:1, 0:KP], axis=AX.X)
        nc.sync.dma_start(out_ap[0:1, 0:1], qtot[0:1, 0:1])

    nc.compile()
    return nc


def _get_program():
    global _PROGRAM
    if _PROGRAM is None:
        _PROGRAM = _build_program()
    return _PROGRAM


def _core0_prepend():
    """2000 synthetic history rows encoding the global init (A,B)=(0,EPS).

    All-zero rows leave the scan at (0,0); the last two rows carry returns
    r1, r2 with r2 = -c*r1 so the A contribution cancels to ~0, while
    eta*(c*r1^2 + r2^2) ~ EPS supplies the B seed.
    """
    w = np.zeros((PRE, NA), NF32)
    nr = np.zeros((PRE, NA), NF32)
    c = CDEC
    r1 = NF32(np.sqrt(EPS / (ETA * (float(c) + float(c) ** 2))))
    r2 = NF32(-(c * r1))
    w[PRE - 2, 0] = NF32(1.0)
    nr[PRE - 2, 0] = r1
    w[PRE - 1, 0] = NF32(1.0)
    nr[PRE - 1, 0] = r2
    return w, nr


def _interleave16(arr):
    # [COLS*KP, NA] f32 -> [KP+1, COLS*NA] fp16, t = col*KP + p; row KP = pad
    out = np.zeros((KP + 1, COLS * NA), NF16)
    out[:KP] = np.ascontiguousarray(
        arr.reshape(COLS, KP, NA).transpose(1, 0, 2).astype(NF16)
    ).reshape(KP, COLS * NA)
    return out


def _make_in_maps(weights, nr):
    weights = np.asarray(weights, dtype=NF32)
    nr = np.asarray(nr, dtype=NF32)
    pre_w, pre_nr = _core0_prepend()
    in_maps = []
    for m in range(N_CORES):
        s = m * OWN
        if m == 0:
            wm = np.concatenate([pre_w, weights[:OWN]])
            rm = np.concatenate([pre_nr, nr[:OWN]])
        else:
            wm = weights[s - PRE:s + OWN]
            rm = nr[s - PRE:s + OWN]
        in_maps.append({"w": _interleave16(wm), "nr": _interleave16(rm)})
    return in_maps


def _run(in_maps, **kwargs):
    nc = _get_program()
    return run_bass_kernel_spmd(nc, in_maps, core_ids=list(range(N_CORES)), **kwargs)


def kernel(weights, next_returns):
    in_maps = _make_in_maps(weights, next_returns)
    res = _run(in_maps)
    total = np.sum(
        np.array([res.results[m]["out"][0, 0] for m in range(N_CORES)], NF32),
        dtype=NF32,
    )
    return NF32(NF32(ETA) * total / NF32(B_TOTAL))


# revision 10
# speedup vs baseline: 1.4119x; 1.0219x over previous
"""Trainium2 Bass kernel for nn_DSRLossStateless (DSR loss, stateless).

loss = -sum_t(D_t)/B where D_t comes from an eta-EMA pair (A,B) over
portfolio returns R_t = sum_a w[t,a]*nr[t,a].

v3 strategy (8 cores, batch-sharded, interleaved layout):
  - Host: fp16 inputs; core m owns 250k rows + 2000 preceding rows
    (history; synthetic EPS-seed rows for core 0). Rows are interleaved
    so SBUF partition p holds times t === p (mod 125): tile [125, 2016]
    with t = col*125 + p. All per-partition DMA stays contiguous.
  - Device, per column-chunk (4 chunks x 504 cols):
      product w*nr (fp16, DVE 2x mode) -> pairwise tree-sum to R (fp16)
      -> R2 = Square(R) on ACT
      -> PE matmuls with triangular geometric matrices L',U' compute the
         125-tap shifted FIR: Y'[p,n] = eta * sum_{j>=1} c^(j-1) R[t-j]
         contributions within the column, accumulated f32 in PSUM
      -> DVE scan with decay c^125 along columns turns Y' into exact
         A_prev[t], B_prev[t] (no carries, no tail correction)
      -> D-chain (f32, split DVE/ACT/GpSimd) accumulates
         q = negn/var^1.5 per chunk, overlapped with the DMA stream.
  - Final: per-partition q sums -> partition gather -> scalar out.
    Host: loss = eta * sum(core outs) / B.
"""

import sys

sys.path.insert(0, "/opt/trn_rl_repo")

import numpy as np

import concourse.bass as bass
import concourse.bacc as bacc
import concourse.tile as tile
from concourse import mybir
from concourse.bass_utils import run_bass_kernel_spmd
from contextlib import ExitStack

F32 = mybir.dt.float32
F16 = mybir.dt.float16
NF32 = np.float32
NF16 = np.float16

N_CORES = 8
NA = 16                 # assets
KP = 125                # partitions (time interleave stride)
PRE_COLS = 16           # prepend history columns
NC_COLS = 2000          # owned columns
COLS = NC_COLS + PRE_COLS   # 2016
OWN = KP * NC_COLS      # 250000 rows per core
PRE = KP * PRE_COLS     # 2000 history rows
B_TOTAL = N_CORES * OWN
CH = 4                  # column chunks
TC = COLS // CH         # 504 columns per chunk
FW = TC * NA            # 8064 fp16 elems per partition per chunk
ETA = 0.01
EPS = 1e-8
CDEC = NF32(1.0 - ETA)  # 0.99
C125 = NF32(float(CDEC) ** KP)

AL = mybir.AluOpType
AF = mybir.ActivationFunctionType
AX = mybir.AxisListType

_PROGRAM = None


def _fir_matrices():
    """lhsT-layout [k, m] FIR matrices with eta folded in.

    A_prev[m, n] = c125*A_prev[m, n-1] + sum_k L[k,m] R[k,n] + U[k,m] R[k,n-1]
    where L[k,m] = eta*c^(m-1-k) for k <= m-1, U[k,m] = eta*c^(125+m-1-k)
    for k >= m.
    """
    k = np.arange(KP)
    m = np.arange(KP)
    c = float(CDEC)
    Lm = np.where(k[:, None] <= m[None, :] - 1,
                  c ** (m[None, :] - 1 - k[:, None]), 0.0)
    Um = np.where(k[:, None] >= m[None, :],
                  c ** (KP + m[None, :] - 1 - k[:, None]), 0.0)
    return (ETA * Lm).astype(NF16), (ETA * Um).astype(NF16)


def _build_program():
    nc = bacc.Bacc("TRN2", target_bir_lowering=False, debug=False)

    # loads span 126 partitions (row 125 is zero padding): 126-row transfers
    # spread across 14 DMA engines; 125-row ones collapse to 5.
    w_ap = nc.dram_tensor("w", [KP + 1, COLS * NA], F16, kind="ExternalInput").ap()
    nr_ap = nc.dram_tensor("nr", [KP + 1, COLS * NA], F16, kind="ExternalInput").ap()
    out_ap = nc.dram_tensor("out", [1, 1], F32, kind="ExternalOutput").ap()

    Lnp, Unp = _fir_matrices()
    L_dram = nc.inline_tensor(np.ascontiguousarray(Lnp), name="Lfir")
    U_dram = nc.inline_tensor(np.ascontiguousarray(Unp), name="Ufir")

    with tile.TileContext(nc) as tc, ExitStack() as ctx:
        pers = ctx.enter_context(tc.tile_pool(name="pers", bufs=1))
        queues = [nc.sync, nc.scalar, nc.gpsimd]
        loadp = ctx.enter_context(tc.tile_pool(name="load", bufs=2))
        prodp = ctx.enter_context(tc.tile_pool(name="prod", bufs=2))
        dchp = ctx.enter_context(tc.tile_pool(name="dch", bufs=2))
        psump = ctx.enter_context(tc.psum_pool(name="ps", bufs=2))

        Lt = pers.tile([KP, KP], F16, tag="Lt")
        Ut = pers.tile([KP, KP], F16, tag="Ut")
        R = pers.tile([KP, 1 + COLS], F16, tag="R")
        R2 = pers.tile([KP, 1 + COLS], F16, tag="R2")
        Aprev = pers.tile([KP, COLS], F32, tag="Aprev")
        Bprev = pers.tile([KP, COLS], F32, tag="Bprev")
        CD = pers.tile([KP, TC], F32, tag="CD")
        qs = pers.tile([KP, CH], F32, tag="qs")
        qp = pers.tile([KP, 1], F32, tag="qp")
        qrow = pers.tile([1, KP], F32, tag="qrow")
        qtot = pers.tile([1, 1], F32, tag="qtot")
        epsc = pers.tile([KP, 1], F32, tag="epsc")

        nc.sync.dma_start(Lt[:], L_dram.ap())
        nc.sync.dma_start(Ut[:], U_dram.ap())
        nc.gpsimd.memset(CD[:, :], float(C125))
        nc.gpsimd.memset(epsc[:, :], EPS)
        nc.gpsimd.memset(R[:, 0:1], 0.0)
        nc.gpsimd.memset(R2[:, 0:1], 0.0)
        # pin ACT tables (Square/Sqrt/Copy) before the pipeline
        nc.gpsimd.memset(qtot[0:1, 0:1], 1.0)
        nc.scalar.activation(qtot[0:1, 0:1], qtot[0:1, 0:1], AF.Sqrt)

        for k in range(CH):
            ks, ke = k * TC, (k + 1) * TC
            rs = slice(1 + ks, 1 + ke)   # R/R2 slices (col 0 is the pad)
            sh = slice(ks, ke)           # shifted R/R2 slices
            cs = slice(ks, ke)           # Aprev/Bprev/D-chain slices

            wt = loadp.tile([KP + 1, FW], F16, tag="wt")
            rt = loadp.tile([KP + 1, FW], F16, tag="rt")
            queues[(2 * k) % 3].dma_start(wt[:], w_ap[:, ks * NA:ke * NA])
            queues[(2 * k + 1) % 3].dma_start(rt[:], nr_ap[:, ks * NA:ke * NA])

            # product (in-place) + pairwise tree rowsum (fp16, DVE 2x)
            u1 = prodp.tile([KP, TC * 8], F16, tag="u1")
            nc.vector.tensor_mul(wt[0:KP, :], wt[0:KP, :], rt[0:KP, :])
            p3 = wt[0:KP, :].rearrange("p (t a) -> p t a", a=16)
            u1v = u1[:].rearrange("p (t a) -> p t a", a=8)
            nc.vector.tensor_add(u1v[:, :, :], p3[:, :, 0:8], p3[:, :, 8:16])
            nc.vector.tensor_add(u1v[:, :, 0:4], u1v[:, :, 0:4], u1v[:, :, 4:8])
            nc.vector.tensor_add(u1v[:, :, 0:2], u1v[:, :, 0:2], u1v[:, :, 2:4])
            nc.vector.tensor_add(R[:, rs], u1v[:, :, 0], u1v[:, :, 1])
            nc.scalar.activation(R2[:, rs], R[:, rs], AF.Square)

            # FIR matmuls into PSUM (f32 accumulate)
            YA = psump.tile([KP, TC], F32, tag="YA")
            YB = psump.tile([KP, TC], F32, tag="YB")
            nc.tensor.matmul(YA[:], lhsT=Lt[:], rhs=R[:, rs], start=True, stop=False)
            nc.tensor.matmul(YB[:], lhsT=Lt[:], rhs=R2[:, rs], start=True, stop=False)
            nc.tensor.matmul(YA[:], lhsT=Ut[:], rhs=R[:, sh], start=False, stop=True)
            nc.tensor.matmul(YB[:], lhsT=Ut[:], rhs=R2[:, sh], start=False, stop=True)

            # exact A_prev/B_prev via c^125-decay scans along columns
            initA = 0.0 if k == 0 else Aprev[:, ks - 1:ks]
            initB = 0.0 if k == 0 else Bprev[:, ks - 1:ks]
            nc.vector.tensor_tensor_scan(
                out=Aprev[:, cs], data0=CD[:, :], data1=YA[:],
                initial=initA, op0=AL.mult, op1=AL.add,
            )
            nc.vector.tensor_tensor_scan(
                out=Bprev[:, cs], data0=CD[:, :], data1=YB[:],
                initial=initB, op0=AL.mult, op1=AL.add,
            )

            # D-chain: q = [0.5*A*(R^2+B) - B*R] / var^1.5
            g1 = dchp.tile([KP, TC], F32, tag="g1")
            g3 = dchp.tile([KP, TC], F32, tag="g3")
            t1 = dchp.tile([KP, TC], F32, tag="t1")
            a2 = dchp.tile([KP, TC], F32, tag="a2")
            vv = dchp.tile([KP, TC], F32, tag="vv")
            ss = dchp.tile([KP, TC], F32, tag="ss")
            dd = dchp.tile([KP, TC], F32, tag="dd")
            rc = dchp.tile([KP, TC], F32, tag="rc")
            nc.gpsimd.tensor_add(g1[:], R2[:, rs], Bprev[:, cs])
            nc.gpsimd.tensor_mul(g3[:], Bprev[:, cs], R[:, rs])
            nc.vector.scalar_tensor_tensor(
                out=t1[:], in0=Aprev[:, cs], scalar=0.5, in1=g1[:],
                op0=AL.mult, op1=AL.mult,
            )
            nc.gpsimd.tensor_sub(t1[:], t1[:], g3[:])          # negn
            nc.scalar.activation(a2[:], Aprev[:, cs], AF.Square)
            nc.gpsimd.tensor_sub(vv[:], Bprev[:, cs], a2[:])
            nc.vector.tensor_scalar_max(vv[:], vv[:], EPS)     # var
            nc.scalar.activation(ss[:], vv[:], AF.Sqrt)
            nc.gpsimd.tensor_mul(dd[:], vv[:], ss[:])          # var^1.5
            nc.vector.reciprocal_approx_accurate(rc[:], dd[:], g1[:])
            qa = slice(PRE_COLS, TC) if k == 0 else slice(0, TC)
            nc.vector.scalar_tensor_tensor(
                out=dd[:, qa], in0=t1[:, qa], scalar=1.0, in1=rc[:, qa],
                op0=AL.mult, op1=AL.mult, accum_out=qs[:, k:k + 1],
            )

        # final reduction: per-partition partials -> scalar
        nc.vector.reduce_sum(qp[:, 0:1], qs[:, :], axis=AX.X)
        nc.sync.dma_start(qrow[0:1, 0:KP], qp[0:KP, 0:1])
        nc.vector.reduce_sum(qtot[0:1, 0:1], qrow[0:1, 0:KP], axis=AX.X)
        nc.sync.dma_start(out_ap[0:1, 0:1], qtot[0:1, 0:1])

    nc.compile()
    return nc


def _get_program():
    global _PROGRAM
    if _PROGRAM is None:
        _PROGRAM = _build_program()
    return _PROGRAM


def _core0_prepend():
    """2000 synthetic history rows encoding the global init (A,B)=(0,EPS).

    All-zero rows leave the scan at (0,0); the last two rows carry returns
    r1, r2 with r2 = -c*r1 so the A contribution cancels to ~0, while
    eta*(c*r1^2 + r2^2) ~ EPS supplies the B seed.
    """
    w = np.zeros((PRE, NA), NF32)
    nr = np.zeros((PRE, NA), NF32)
    c = CDEC
    r1 = NF32(np.sqrt(EPS / (ETA * (float(c) + float(c) ** 2))))
    r2 = NF32(-(c * r1))
    w[PRE - 2, 0] = NF32(1.0)
    nr[PRE - 2, 0] = r1
    w[PRE - 1, 0] = NF32(1.0)
    nr[PRE - 1, 0] = r2
    return w, nr


def _interleave16(arr):
    # [COLS*KP, NA] f32 -> [KP+1, COLS*NA] fp16, t = col*KP + p; row KP = pad
    out = np.zeros((KP + 1, COLS * NA), NF16)
    out[:KP] = np.ascontiguousarray(
        arr.reshape(COLS, KP, NA).transpose(1, 0, 2).astype(NF16)
    ).reshape(KP, COLS * NA)
    return out


def _make_in_maps(weights, nr):
    weights = np.asarray(weights, dtype=NF32)
    nr = np.asarray(nr, dtype=NF32)
    pre_w, pre_nr = _core0_prepend()
    in_maps = []
    for m in range(N_CORES):
        s = m * OWN
        if m == 0:
            wm = np.concatenate([pre_w, weights[:OWN]])
            rm = np.concatenate([pre_nr, nr[:OWN]])
        else:
            wm = weights[s - PRE:s + OWN]
            rm = nr[s - PRE:s + OWN]
        in_maps.append({"w": _interleave16(wm), "nr": _interleave16(rm)})
    return in_maps


def _run(in_maps, **kwargs):
    nc = _get_program()
    return run_bass_kernel_spmd(nc, in_maps, core_ids=list(range(N_CORES)), **kwargs)


def kernel(weights, next_returns):
    in_maps = _make_in_maps(weights, next_returns)
    res = _run(in_maps)
    total = np.sum(
        np.array([res.results[m]["out"][0, 0] for m in range(N_CORES)], NF32),
        dtype=NF32,
    )
    return NF32(NF32(ETA) * total / NF32(B_TOTAL))
